# revision 7
# baseline (speedup 1.0000x reference)
"""DiffuseRouter kernel for 8 TRN2 NeuronCores.

Reference computation (enable_time=False, soft_time_routing=True):
    out[b, l, d] = (1/3) * sum_g sum_e expert_emb_g[e, b, l, d]
i.e. a uniform-weighted sum of 28 expert planes per batch element.

Sharding: pure data-parallel over batch B=8 -> one batch element per core.
Each core reads its 28 [256, 1280] f32 planes (36.7 MB), reduces them
on-chip, scales by 1/3, and writes its [256, 1280] output.  No collectives
needed (B == n_cores), which is strictly less traffic than expert-parallel
+ all-reduce.

Implementation notes (from perfetto traces):
  * Loads go through SWDGE (nc.gpsimd): the HWDGE descriptor rings sit on
    SBUF partitions served by SDMA engine 15, so HWDGE loads throttle that
    one engine to ~60% rate and every tile-completion semaphore inherits
    the straggler's pace.  SWDGE spreads descriptor traffic and all 16
    engines run at an identical ~29 GB/s.
  * The SWDGE load also casts f32 -> bf16 in the SDMA datapath, halving
    SBUF-side write traffic.
  * Accumulation runs on the TensorEngine as identity-matmuls into fp32
    PSUM (bf16 inputs, fp32 accumulate), ~1.1 us/plane vs 2.9 us/plane for
    DVE f32 scalar_tensor_tensor -- PE rides far under the DMA stream rate
    instead of pacing it.  Only input rounding to bf16 affects accuracy
    (rel err ~4e-3, well under the 2e-2 gate).
  * DVE only does the five final PSUM -> SBUF x(1/3) copies; stores go on
    the otherwise-idle HWDGE rings.
"""

import numpy as np
import ml_dtypes

import concourse.bacc as bacc
import concourse.tile as tile
from concourse import mybir
from concourse.bass import MemorySpace
from concourse.bass_utils import run_bass_kernel_spmd

N_CORES = 8
E_TOTAL = 28  # 4 + 8 + 16 experts across the 3 granularity levels
L, D = 256, 1280
P = 128  # SBUF partitions
FD = (L // P) * D  # 2560 free-dim elements per partition
NCH = 5  # PSUM chunks: 2560 = 5 x 512 (one PSUM bank each)
CH = FD // NCH
SCALE = 1.0 / 3.0

_NC_CACHE = None


def _build_nc():
    """Build the SPMD Bass program (identical on all 8 cores)."""
    nc = bacc.Bacc(
        "TRN2", target_bir_lowering=False, debug=False, enable_partition_id=False
    )
    x = nc.dram_tensor("x", [E_TOTAL, L, D], mybir.dt.float32, kind="ExternalInput")
    ident = nc.dram_tensor("ident", [P, P], mybir.dt.bfloat16, kind="ExternalInput")
    out = nc.dram_tensor("out", [L, D], mybir.dt.float32, kind="ExternalOutput")

    # [E, 256, 1280] -> [E, 128, 2560]: partition p holds rows 2p, 2p+1
    # (contiguous 10240 B per partition -> fully linear DMA per plane).
    x_t = x.ap().rearrange("e (p a) d -> e p (a d)", a=2)
    out_t = out.ap().rearrange("(p a) d -> p (a d)", a=2)

    with tile.TileContext(nc) as tc:
        with (
            tc.tile_pool(name="in", bufs=6) as pin,
            tc.tile_pool(name="w", bufs=1) as pw,
            tc.tile_pool(name="res", bufs=1) as pres,
            tc.tile_pool(name="ps", bufs=1, space=MemorySpace.PSUM) as pps,
        ):
            idt = pw.tile([P, P], mybir.dt.bfloat16, name="idt", tag="idt")
            nc.sync.dma_start(out=idt[:], in_=ident.ap())
            psum = pps.tile([P, NCH, CH], mybir.dt.float32, name="psum", tag="psum")
            res = pres.tile([P, FD], mybir.dt.float32, name="res", tag="res")

            last = E_TOTAL - 1
            for e in range(E_TOTAL):
                if e < last:
                    # Full-plane casting load (f32 HBM -> bf16 SBUF).
                    t = pin.tile([P, FD], mybir.dt.bfloat16)
                    nc.gpsimd.dma_start(out=t[:], in_=x_t[e])
                    chunks = [t[:, c * CH : (c + 1) * CH] for c in range(NCH)]
                else:
                    # Last plane: per-chunk loads so each final matmul (and
                    # the copy/store chain behind it) starts as soon as its
                    # own 512 columns land.
                    chunks = []
                    for c in range(NCH):
                        qt = pin.tile(
                            [P, CH], mybir.dt.bfloat16, name=f"tq{c}", tag=f"tq{c}"
                        )
                        nc.gpsimd.dma_start(
                            out=qt[:], in_=x_t[e][:, c * CH : (c + 1) * CH]
                        )
                        chunks.append(qt[:])
                for c in range(NCH):
                    # psum[:, c, :] (+)= I^T @ t[:, chunk c]
                    nc.tensor.matmul(
                        psum[:, c],
                        idt[:],
                        chunks[c],
                        start=(e == 0),
                        stop=(e == last),
                    )
                    if e == last:
                        # res = psum * 1/3, then store this chunk.
                        sl = slice(c * CH, (c + 1) * CH)
                        nc.vector.tensor_scalar_mul(res[:, sl], psum[:, c], SCALE)
                        eng = nc.sync if c % 2 == 0 else nc.scalar
                        eng.dma_start(out=out_t[:, sl], in_=res[:, sl])
    nc.compile()
    return nc


def _get_nc():
    global _NC_CACHE
    if _NC_CACHE is None:
        _NC_CACHE = _build_nc()
    return _NC_CACHE


def _run(inputs, trace=False, trace_kwargs=None):
    e0 = np.asarray(inputs["expert_emb_0"], dtype=np.float32)
    e1 = np.asarray(inputs["expert_emb_1"], dtype=np.float32)
    e2 = np.asarray(inputs["expert_emb_2"], dtype=np.float32)
    B = e0.shape[1]
    assert B == N_CORES, f"expected B == {N_CORES}, got {B}"

    ident = np.eye(P, dtype=ml_dtypes.bfloat16)
    in_maps = []
    for b in range(B):
        xb = np.concatenate([e0[:, b], e1[:, b], e2[:, b]], axis=0)
        in_maps.append({"x": np.ascontiguousarray(xb), "ident": ident})

    kw = {}
    if trace:
        kw["trace"] = True
        if trace_kwargs:
            kw.update(trace_kwargs)
    try:
        res = run_bass_kernel_spmd(_get_nc(), in_maps, list(range(N_CORES)), **kw)
    except Exception:
        # One retry: transient device errors (e.g. NRT unrecoverable after a
        # prior wedged run) usually clear on re-dispatch.
        res = run_bass_kernel_spmd(_get_nc(), in_maps, list(range(N_CORES)), **kw)
    out = np.stack([res.results[b]["out"] for b in range(B)], axis=0)
    return out.astype(np.float32, copy=False), res


def kernel(**inputs) -> np.ndarray:
    out, _ = _run(inputs, trace=False)
    return out


# revision 8
# speedup vs baseline: 1.1679x; 1.1679x over previous
"""DiffuseRouter kernel for 8 TRN2 NeuronCores.

Reference computation (enable_time=False, soft_time_routing=True):
    out[b, l, d] = (1/3) * sum_g sum_e expert_emb_g[e, b, l, d]
i.e. a uniform-weighted sum of 28 expert planes per batch element.

Sharding: pure data-parallel over batch B=8 -> one batch element per core.
Each core reads its 28 [256, 1280] f32 planes (36.7 MB), reduces them
on-chip, scales by 1/3, and writes its [256, 1280] output.  No collectives
needed (B == n_cores), which is strictly less traffic than expert-parallel
+ all-reduce.

Implementation notes (from perfetto traces):
  * Loads go through SWDGE (nc.gpsimd): the HWDGE descriptor rings sit on
    SBUF partitions served by SDMA engine 15, so HWDGE loads throttle that
    one engine to ~60% rate and every tile-completion semaphore inherits
    the straggler's pace.  SWDGE spreads descriptor traffic and all 16
    engines run at an identical ~29 GB/s.
  * The SWDGE load also casts f32 -> bf16 in the SDMA datapath, halving
    SBUF-side write traffic.
  * Accumulation runs on the TensorEngine as identity-matmuls into fp32
    PSUM (bf16 inputs, fp32 accumulate), ~1.1 us/plane vs 2.9 us/plane for
    DVE f32 scalar_tensor_tensor -- PE rides far under the DMA stream rate
    instead of pacing it.  Only input rounding to bf16 affects accuracy
    (rel err ~4e-3, well under the 2e-2 gate).
  * DVE only does the five final PSUM -> SBUF x(1/3) copies; stores go on
    the otherwise-idle HWDGE rings.
"""

import numpy as np
import ml_dtypes

import concourse.bacc as bacc
import concourse.tile as tile
from concourse import mybir
from concourse.bass import MemorySpace
from concourse.bass_utils import run_bass_kernel_spmd

N_CORES = 8
E_TOTAL = 28  # 4 + 8 + 16 experts across the 3 granularity levels
L, D = 256, 1280
P = 128  # SBUF partitions
FD = (L // P) * D  # 2560 free-dim elements per partition
NCH = 5  # PSUM chunks: 2560 = 5 x 512 (one PSUM bank each)
CH = FD // NCH
SCALE = 1.0 / 3.0

_NC_CACHE = None


def _build_nc():
    """Build the SPMD Bass program (identical on all 8 cores)."""
    nc = bacc.Bacc(
        "TRN2", target_bir_lowering=False, debug=False, enable_partition_id=False
    )
    x = nc.dram_tensor("x", [E_TOTAL, L, D], mybir.dt.float32, kind="ExternalInput")
    ident = nc.dram_tensor("ident", [P, P], mybir.dt.bfloat16, kind="ExternalInput")
    out = nc.dram_tensor("out", [L, D], mybir.dt.float32, kind="ExternalOutput")

    # [E, 256, 1280] -> [E, 128, 2560]: partition p holds rows 2p, 2p+1
    # (contiguous 10240 B per partition -> fully linear DMA per plane).
    x_t = x.ap().rearrange("e (p a) d -> e p (a d)", a=2)
    out_t = out.ap().rearrange("(p a) d -> p (a d)", a=2)

    with tile.TileContext(nc) as tc:
        with (
            tc.tile_pool(name="in", bufs=6) as pin,
            tc.tile_pool(name="w", bufs=1) as pw,
            tc.tile_pool(name="res", bufs=1) as pres,
            tc.tile_pool(name="ps", bufs=1, space=MemorySpace.PSUM) as pps,
        ):
            idt = pw.tile([P, P], mybir.dt.bfloat16, name="idt", tag="idt")
            nc.sync.dma_start(out=idt[:], in_=ident.ap())
            psum = pps.tile([P, NCH, CH], mybir.dt.float32, name="psum", tag="psum")
            res = pres.tile([P, FD], mybir.dt.float32, name="res", tag="res")

            last = E_TOTAL - 1
            for e in range(E_TOTAL):
                # Full-plane casting load (f32 HBM -> bf16 SBUF).
                t = pin.tile([P, FD], mybir.dt.bfloat16)
                nc.gpsimd.dma_start(out=t[:], in_=x_t[e])
                chunks = [t[:, c * CH : (c + 1) * CH] for c in range(NCH)]
                for c in range(NCH):
                    # psum[:, c, :] (+)= I^T @ t[:, chunk c]
                    nc.tensor.matmul(
                        psum[:, c],
                        idt[:],
                        chunks[c],
                        start=(e == 0),
                        stop=(e == last),
                    )
                    if e == last:
                        # res = psum * 1/3, then store this chunk.
                        sl = slice(c * CH, (c + 1) * CH)
                        nc.vector.tensor_scalar_mul(res[:, sl], psum[:, c], SCALE)
                        eng = nc.sync if c % 2 == 0 else nc.scalar
                        eng.dma_start(out=out_t[:, sl], in_=res[:, sl])
    nc.compile()
    return nc


def _get_nc():
    global _NC_CACHE
    if _NC_CACHE is None:
        _NC_CACHE = _build_nc()
    return _NC_CACHE


def _run(inputs, trace=False, trace_kwargs=None):
    e0 = np.asarray(inputs["expert_emb_0"], dtype=np.float32)
    e1 = np.asarray(inputs["expert_emb_1"], dtype=np.float32)
    e2 = np.asarray(inputs["expert_emb_2"], dtype=np.float32)
    B = e0.shape[1]
    assert B == N_CORES, f"expected B == {N_CORES}, got {B}"

    ident = np.eye(P, dtype=ml_dtypes.bfloat16)
    in_maps = []
    for b in range(B):
        xb = np.concatenate([e0[:, b], e1[:, b], e2[:, b]], axis=0)
        in_maps.append({"x": np.ascontiguousarray(xb), "ident": ident})

    kw = {}
    if trace:
        kw["trace"] = True
        if trace_kwargs:
            kw.update(trace_kwargs)
    try:
        res = run_bass_kernel_spmd(_get_nc(), in_maps, list(range(N_CORES)), **kw)
    except Exception:
        # One retry: transient device errors (e.g. NRT unrecoverable after a
        # prior wedged run) usually clear on re-dispatch.
        res = run_bass_kernel_spmd(_get_nc(), in_maps, list(range(N_CORES)), **kw)
    out = np.stack([res.results[b]["out"] for b in range(B)], axis=0)
    return out.astype(np.float32, copy=False), res


def kernel(**inputs) -> np.ndarray:
    out, _ = _run(inputs, trace=False)
    return out
